# revision 19
# baseline (speedup 1.0000x reference)
"""LSTM kernel for Trainium2 (Bass/Tile), SPMD over 8 NeuronCores.

Problem: B=128, S=1024, D=256, H=512, C=10 LSTM; output = final hidden state
projected to C classes -> [B, C].

Sharding: data-parallel over batch (16 per core); weights replicated;
recurrence local per shard (no collectives).

Per-core design (fused single pass; x read once, proj never leaves chip):
  - x-projections computed in m-tiles of 8 timesteps (M=128 = full PE
    width) a few steps ahead of the recurrence, result cast to bf16 into
    an SBUF ring -- no DRAM proj round-trip.
  - Per step, gate preactivations live in one PSUM tile [16, 2048]
    (4 banks: g|i|f|o). Each bank's accumulation group: an identity
    matmul injects proj (+bias, folded via phase-1's ones-row) with
    start=True, then 4 h @ Wh matmuls accumulate (bf16, N=512).
  - Banks are processed f,g,i,o so ACT/DVE for early gates overlap the
    later banks' matmuls; o last (shortest post-MM path).
  - Tail: cell update in bf16 [16, 512] (DVE 2x mode); then PE-transpose
    c and o quarters into [128, 64] tiles; hT = tanh(cT) * oT written
    directly in the stationary layout hT[:, 16k:16k+16] (no h transpose).
"""

import numpy as np

S, B, D, H, C = 1024, 128, 256, 512, 10
NCORES = 8
BC = B // NCORES          # batch per core (16)
G4 = 4 * H                # fused gate width (2048)
NKH = H // 128            # 4 k-tiles for h
NKX = D // 128 + 1        # 2 k-tiles for x + 1 bias(ones) tile
TPM = 8                   # timesteps per phase-1 m-tile (128/BC)
LOOKAHEAD = 2             # m-tiles of proj lookahead
# gate memory order [g,i,f,o]; processing order f,g,i,o (o last)
PROC = [2, 0, 1, 3]


def _build_nc(s_total: int):
    import concourse.bass as bass
    import concourse.mybir as mybir
    import concourse.tile as tile
    from concourse import bacc
    from concourse.masks import make_identity

    f32 = mybir.dt.float32
    bf16 = mybir.dt.bfloat16
    AF = mybir.ActivationFunctionType

    n_mt = (s_total + TPM - 1) // TPM
    assert s_total % TPM == 0

    nc = bacc.Bacc(
        "TRN2",
        target_bir_lowering=False,
        debug=False,
        enable_asserts=False,
        num_devices=NCORES,
    )

    xT_d = nc.dram_tensor("xT", [n_mt, 128, NKX * 128], bf16, kind="ExternalInput").ap()
    Wx_d = nc.dram_tensor("Wx", [128, NKX * G4], bf16, kind="ExternalInput").ap()
    Wh_d = nc.dram_tensor("Wh", [128, NKH * G4], bf16, kind="ExternalInput").ap()
    i128_d = nc.dram_tensor("ident128", [128, 128], bf16, kind="ExternalInput").ap()
    Wp_d = nc.dram_tensor("Wp", [NKH, 128, C], bf16, kind="ExternalInput").ap()
    bp_d = nc.dram_tensor("bp", [BC, C], f32, kind="ExternalInput").ap()
    out_d = nc.dram_tensor("out", [BC, C], f32, kind="ExternalOutput").ap()

    with tile.TileContext(nc) as tc:
        with (
            tc.tile_pool(name="const", bufs=1) as const,
            tc.tile_pool(name="state", bufs=1) as state,
            tc.tile_pool(name="xring", bufs=3) as xring,
            tc.tile_pool(name="pring", bufs=LOOKAHEAD + 2) as pring,
            tc.tile_pool(name="p1ps", bufs=2, space="PSUM") as p1ps,
            tc.tile_pool(name="gbank", bufs=1, space="PSUM") as gbank,
            tc.tile_pool(name="tbank", bufs=1, space="PSUM") as tbank,
            tc.tile_pool(name="work", bufs=2) as work,
        ):
            Wx_sb = const.tile([128, NKX * G4], bf16)
            nc.sync.dma_start(Wx_sb[:], Wx_d[:])
            Wh_sb = const.tile([128, NKH * G4], bf16)
            nc.sync.dma_start(Wh_sb[:], Wh_d[:])
            i128_sb = const.tile([128, 128], bf16)
            nc.sync.dma_start(i128_sb[:], i128_d[:])
            Wp_sb = const.tile([128, NKH * C], bf16)
            nc.sync.dma_start(
                Wp_sb[:].rearrange("p (k c) -> p k c", k=NKH),
                Wp_d.rearrange("k p c -> p k c"),
            )
            bp_sb = const.tile([BC, C], f32)
            nc.sync.dma_start(bp_sb[:], bp_d[:])
            ident = const.tile([128, 128], f32)
            make_identity(nc, ident[:])

            # state: hT bf16 [128, NKH*BC] (hT[:, 16k:16k+16] = h k-tile),
            # c fp32 [16, 512], both ping-pong.
            hT = [state.tile([128, NKH * BC], bf16, tag=f"hT{i}", name=f"hT{i}") for i in range(2)]
            cs = [state.tile([BC, H], f32, tag=f"c{i}", name=f"c{i}") for i in range(2)]

            xtiles, ptiles = {}, {}

            def dma_xtile(m):
                xtiles[m] = xring.tile([128, NKX * 128], bf16, tag="xt", name=f"xt{m}")
                nc.sync.dma_start(xtiles[m][:], xT_d[m])

            def phase1_chunk(m, jj):
                """One gate-chunk (512 cols) of m-tile m: 3 MMs + cast-copy."""
                if jj == 0:
                    ptiles[m] = pring.tile([128, G4], bf16, tag="proj", name=f"proj{m}")
                    if m + 1 < n_mt and m + 1 not in xtiles:
                        dma_xtile(m + 1)
                xt = xtiles[m]
                ps = p1ps.tile([128, 512], f32, tag="p1", name=f"p1_{m}_{jj}")
                for k in range(NKX):
                    nc.tensor.matmul(
                        ps[:],
                        lhsT=xt[:, k * 128:(k + 1) * 128],
                        rhs=Wx_sb[:, k * G4 + jj * 512: k * G4 + (jj + 1) * 512],
                        start=(k == 0),
                        stop=(k == NKX - 1),
                    )
                dst = ptiles[m][:, jj * 512:(jj + 1) * 512]
                if jj % 2 == 0:
                    nc.vector.tensor_copy(dst, ps[:])
                else:
                    nc.scalar.copy(dst, ps[:])

            def inject(t, banks):
                """Start each gate bank's accumulation with proj(+bias)."""
                m, tt = t // TPM, t % TPM
                for j in PROC:
                    nc.tensor.matmul(
                        banks[j][:],
                        lhsT=i128_sb[:, BC * tt:BC * (tt + 1)],
                        rhs=ptiles[m][:, j * 512:(j + 1) * 512],
                        start=True,
                        stop=(t == 0),
                    )

            # ---------------- preamble ----------------
            for m in range(min(LOOKAHEAD + 1, n_mt)):
                dma_xtile(m)
            for m in range(min(LOOKAHEAD, n_mt)):
                for jj in range(4):
                    phase1_chunk(m, jj)

            banks = [gbank.tile([BC, 512], f32, tag=f"gb{j}", name=f"gb{j}") for j in range(4)]
            inject(0, banks)

            p1_queue = []  # pending (m, jj) phase-1 chunks, 1 emitted per step
            for m in range(LOOKAHEAD, n_mt):
                for jj in range(4):
                    p1_queue.append((m, jj))

            qi = 0
            for t in range(s_total):
                ping = t % 2

                # h @ Wh accumulation, bank-major (f, g, i, o)
                if t > 0:
                    hprev = hT[(t + 1) % 2]
                    for j in PROC:
                        for k in range(NKH):
                            nc.tensor.matmul(
                                banks[j][:],
                                lhsT=hprev[:, BC * k:BC * (k + 1)],
                                rhs=Wh_sb[:, k * G4 + j * 512: k * G4 + (j + 1) * 512],
                                start=False,
                                stop=(k == NKH - 1),
                            )

                # gate activations per bank, in processing order
                gf = work.tile([BC, H], f32, tag="gf", name="gf")
                nc.scalar.activation(gf[:], banks[2][:], AF.Sigmoid)
                gg = work.tile([BC, H], f32, tag="gg", name="gg")
                nc.scalar.activation(gg[:], banks[0][:], AF.Tanh)
                gi_ = work.tile([BC, H], f32, tag="gi", name="gi")
                nc.scalar.activation(gi_[:], banks[1][:], AF.Sigmoid)
                go = work.tile([BC, H], f32, tag="go", name="go")
                nc.scalar.activation(go[:], banks[3][:], AF.Sigmoid)

                # cell update (fp32): c = g*i + c*f
                prod = work.tile([BC, H], f32, tag="prod", name="prod")
                nc.vector.tensor_mul(prod[:], gg[:], gi_[:])
                if t > 0:
                    cf = work.tile([BC, H], f32, tag="cf", name="cf")
                    nc.vector.tensor_mul(cf[:], cs[(t + 1) % 2][:], gf[:])
                    nc.vector.tensor_add(cs[ping][:], prod[:], cf[:])
                else:
                    nc.vector.tensor_copy(cs[ping][:], prod[:])

                # prefill next step's banks (reuse same 4 PSUM tiles)
                if t + 1 < s_total:
                    banks = [gbank.tile([BC, 512], f32, tag=f"gb{j}", name=f"gb{j}_{t + 1}") for j in range(4)]
                    inject(t + 1, banks)

                # transposes: o then c quarters -> [128, BC] tiles
                oT = tbank.tile([128, NKH * BC], f32, tag="oT", name="oT")
                cT = tbank.tile([128, NKH * BC], f32, tag="cT", name="cT")
                for k in range(NKH):
                    nc.tensor.transpose(
                        oT[:, BC * k:BC * (k + 1)],
                        go[:, 128 * k:128 * (k + 1)],
                        ident[0:BC, 0:BC],
                    )
                for k in range(NKH):
                    nc.tensor.transpose(
                        cT[:, BC * k:BC * (k + 1)],
                        cs[ping][:, 128 * k:128 * (k + 1)],
                        ident[0:BC, 0:BC],
                    )
                # one phase-1 chunk per 2 steps fills the PE gap here
                if t % 2 == 0 and qi < len(p1_queue):
                    phase1_chunk(*p1_queue[qi])
                    qi += 1

                thT = work.tile([128, NKH * BC], f32, tag="thT", name="thT")
                nc.scalar.activation(thT[:], cT[:], AF.Tanh)
                nc.vector.tensor_mul(hT[ping][:], thT[:], oT[:])

            # ---------------- final projection ----------------
            fin = (s_total + 1) % 2
            pso = p1ps.tile([BC, C], f32, tag="p1", name="pso")
            for k in range(NKH):
                nc.tensor.matmul(
                    pso[:],
                    lhsT=hT[fin][:, BC * k:BC * (k + 1)],
                    rhs=Wp_sb[:, k * C:(k + 1) * C],
                    start=(k == 0),
                    stop=(k == NKH - 1),
                )
            res = work.tile([BC, C], f32, tag="res", name="res")
            nc.vector.tensor_add(res[:], pso[:], bp_sb[:])
            nc.sync.dma_start(out_d[:], res[:])

    nc.compile()
    return nc


def _prep_shared_inputs(Wgx, Wix, Wfx, Wox, Wgh, Wih, Wfh, Woh, bg, bi, bf, bo, Wph, bp):
    import ml_dtypes
    bf16 = ml_dtypes.bfloat16
    Wx_all = np.concatenate([Wgx, Wix, Wfx, Wox], axis=1).astype(np.float32)  # [D, G4]
    b_all = np.concatenate([bg, bi, bf, bo]).astype(np.float32)               # [G4]
    Wh_all = np.concatenate([Wgh, Wih, Wfh, Woh], axis=1).astype(np.float32)  # [H, G4]

    Wx = np.zeros((NKX, 128, G4), dtype=np.float32)
    Wx[:D // 128] = Wx_all.reshape(D // 128, 128, G4)
    Wx[NKX - 1, 0, :] = b_all                     # bias row (pairs with ones-row)
    Wx = np.ascontiguousarray(Wx.transpose(1, 0, 2)).reshape(128, NKX * G4)
    Wh = np.ascontiguousarray(Wh_all.reshape(NKH, 128, G4).transpose(1, 0, 2)).reshape(128, NKH * G4)
    Wp = np.ascontiguousarray(Wph.reshape(NKH, 128, C))
    bpr = np.broadcast_to(bp.astype(np.float32), (BC, C)).copy()
    return (Wx.astype(bf16), Wh.astype(bf16), np.eye(128, dtype=np.float32).astype(bf16),
            Wp.astype(bf16), bpr)


def _prep_core_inputs(x, core, s_total):
    """xT[m, p, k*128 + col], col = tt*16 + b: stationary x tiles + ones row."""
    import ml_dtypes
    n_mt = s_total // TPM
    b0 = core * BC
    xc = np.asarray(x[b0:b0 + BC, :s_total, :], dtype=np.float32)   # [BC, s, D]
    a = np.ascontiguousarray(xc.transpose(2, 1, 0))                 # [D, s, BC]
    a = a.reshape(D // 128, 128, n_mt, TPM, BC)                     # [k, p, m, tt, b]
    a = a.transpose(2, 1, 0, 3, 4).reshape(n_mt, 128, D // 128, TPM * BC)
    xT = np.zeros((n_mt, 128, NKX, 128), dtype=np.float32)
    xT[:, :, :D // 128, :] = a
    xT[:, 0, NKX - 1, :] = 1.0                                      # ones row
    return {"xT": np.ascontiguousarray(xT).reshape(n_mt, 128, NKX * 128).astype(ml_dtypes.bfloat16)}


_NC_CACHE = {}


def _get_nc(s_total):
    if s_total not in _NC_CACHE:
        _NC_CACHE[s_total] = _build_nc(s_total)
    return _NC_CACHE[s_total]


def kernel(x, Wgx, Wix, Wfx, Wox, Wgh, Wih, Wfh, Woh, bg, bi, bf, bo, Wph, bp,
           _s_total=S, _trace=False, _trace_kwargs=None):
    from concourse import bass_utils

    x = np.asarray(x, dtype=np.float32)
    args = [np.asarray(a, dtype=np.float32) for a in
            (Wgx, Wix, Wfx, Wox, Wgh, Wih, Wfh, Woh, bg, bi, bf, bo, Wph, bp)]
    Wx, Wh, i16, Wp, bpr = _prep_shared_inputs(*args)

    nc = _get_nc(_s_total)
    in_maps = []
    for core in range(NCORES):
        m = _prep_core_inputs(x, core, _s_total)
        m.update({"Wx": Wx, "Wh": Wh, "ident128": i16, "Wp": Wp, "bp": bpr})
        in_maps.append(m)

    kw = {}
    if _trace:
        kw["trace"] = True
        kw.update(_trace_kwargs or {})
    res = bass_utils.run_bass_kernel_spmd(nc, in_maps, core_ids=list(range(NCORES)), **kw)
    out = np.concatenate([res.results[c]["out"] for c in range(NCORES)], axis=0)
    if _trace:
        kernel._last_results = res
    return out


def _sim_selftest(s_total=32, core=1):
    """CoreSim numerics check on one core vs numpy LSTM (no hardware)."""
    from concourse.bass_interp import CoreSim

    rng = np.random.default_rng(0)
    x = rng.standard_normal((B, s_total, D), dtype=np.float32)
    mk = lambda *s: (rng.standard_normal(s, dtype=np.float32) * 0.06)
    Wgx, Wix, Wfx, Wox = (mk(D, H) for _ in range(4))
    Wgh, Wih, Wfh, Woh = (mk(H, H) for _ in range(4))
    bg, bi, bf, bo = (rng.standard_normal(H).astype(np.float32) * 0.05 for _ in range(4))
    Wph = mk(H, C)
    bp = rng.standard_normal(C).astype(np.float32) * 0.05

    def ref_np(xc):
        sig = lambda v: 1.0 / (1.0 + np.exp(-v))
        h = np.zeros((xc.shape[0], H), np.float32)
        c = np.zeros((xc.shape[0], H), np.float32)
        for t in range(s_total):
            xt = xc[:, t, :]
            g = np.tanh(xt @ Wgx + bg + h @ Wgh)
            i = sig(xt @ Wix + bi + h @ Wih)
            f = sig(xt @ Wfx + bf + h @ Wfh)
            o = sig(xt @ Wox + bo + h @ Woh)
            c = g * i + c * f
            h = np.tanh(c) * o
        return h @ Wph + bp

    args = (Wgx, Wix, Wfx, Wox, Wgh, Wih, Wfh, Woh, bg, bi, bf, bo, Wph, bp)
    Wx, Wh, i16, Wp, bpr = _prep_shared_inputs(*args)
    nc = _build_nc(s_total)

    m = _prep_core_inputs(x, core, s_total)
    m.update({"Wx": Wx, "Wh": Wh, "ident128": i16, "Wp": Wp, "bp": bpr})

    sim = CoreSim(nc)
    for k, v in m.items():
        sim.tensor(k)[:] = v
    sim.simulate(check_with_hw=False)
    got = np.array(sim.tensor("out"))
    want = ref_np(x[core * BC:(core + 1) * BC])
    err = np.abs(got - want).max() / max(np.abs(want).max(), 1e-6)
    print(f"selftest S={s_total}: rel err {err:.3e}")
    assert err < 2e-2, err
    return err


if __name__ == "__main__":
    _sim_selftest(32)


# revision 20
# speedup vs baseline: 1.2307x; 1.2307x over previous
"""LSTM kernel for Trainium2 (Bass/Tile), SPMD over 8 NeuronCores.

Problem: B=128, S=1024, D=256, H=512, C=10 LSTM; output = final hidden state
projected to C classes -> [B, C].

Sharding: data-parallel over batch (16 per core); weights replicated;
recurrence local per shard (no collectives).

Per-core design (fused single pass; x read once, proj never leaves chip):
  - x-projections computed in m-tiles of 8 timesteps (M=128 = full PE
    width) a few steps ahead of the recurrence, result cast to bf16 into
    an SBUF ring -- no DRAM proj round-trip.
  - Per step, gate preactivations live in one PSUM tile [16, 2048]
    (4 banks: g|i|f|o). Each bank's accumulation group: an identity
    matmul injects proj (+bias, folded via phase-1's ones-row) with
    start=True, then 4 h @ Wh matmuls accumulate (bf16, N=512).
  - Banks are processed f,g,i,o so ACT/DVE for early gates overlap the
    later banks' matmuls; o last (shortest post-MM path).
  - Tail: cell update in bf16 [16, 512] (DVE 2x mode); then PE-transpose
    c and o quarters into [128, 64] tiles; hT = tanh(cT) * oT written
    directly in the stationary layout hT[:, 16k:16k+16] (no h transpose).
"""

import numpy as np

S, B, D, H, C = 1024, 128, 256, 512, 10
NCORES = 8
BC = B // NCORES          # batch per core (16)
G4 = 4 * H                # fused gate width (2048)
NKH = H // 128            # 4 k-tiles for h
NKX = D // 128 + 1        # 2 k-tiles for x + 1 bias(ones) tile
TPM = 8                   # timesteps per phase-1 m-tile (128/BC)
LOOKAHEAD = 2             # m-tiles of proj lookahead
# gate memory order [g,i,f,o]; processing order f,g,i,o (o last)
PROC = [2, 0, 1, 3]


def _build_nc(s_total: int):
    import concourse.bass as bass
    import concourse.mybir as mybir
    import concourse.tile as tile
    from concourse import bacc
    from concourse.masks import make_identity

    f32 = mybir.dt.float32
    bf16 = mybir.dt.bfloat16
    AF = mybir.ActivationFunctionType

    n_mt = (s_total + TPM - 1) // TPM
    assert s_total % TPM == 0

    nc = bacc.Bacc(
        "TRN2",
        target_bir_lowering=False,
        debug=False,
        enable_asserts=False,
        num_devices=NCORES,
    )

    xT_d = nc.dram_tensor("xT", [n_mt, 128, NKX * 128], bf16, kind="ExternalInput").ap()
    Wx_d = nc.dram_tensor("Wx", [128, NKX * G4], bf16, kind="ExternalInput").ap()
    Wh_d = nc.dram_tensor("Wh", [128, NKH * G4], bf16, kind="ExternalInput").ap()
    i128_d = nc.dram_tensor("ident128", [128, 128], bf16, kind="ExternalInput").ap()
    Wp_d = nc.dram_tensor("Wp", [NKH, 128, C], bf16, kind="ExternalInput").ap()
    bp_d = nc.dram_tensor("bp", [BC, C], f32, kind="ExternalInput").ap()
    out_d = nc.dram_tensor("out", [BC, C], f32, kind="ExternalOutput").ap()

    with tile.TileContext(nc) as tc:
        with (
            tc.tile_pool(name="const", bufs=1) as const,
            tc.tile_pool(name="state", bufs=1) as state,
            tc.tile_pool(name="xring", bufs=3) as xring,
            tc.tile_pool(name="pring", bufs=LOOKAHEAD + 2) as pring,
            tc.tile_pool(name="p1ps", bufs=1, space="PSUM") as p1ps,
            tc.tile_pool(name="gbank", bufs=1, space="PSUM") as gbank,
            tc.tile_pool(name="tbank", bufs=1, space="PSUM") as tbank,
            tc.tile_pool(name="work", bufs=2) as work,
        ):
            Wx_sb = const.tile([128, NKX * G4], bf16)
            nc.sync.dma_start(Wx_sb[:], Wx_d[:])
            Wh_sb = const.tile([128, NKH * G4], bf16)
            nc.sync.dma_start(Wh_sb[:], Wh_d[:])
            i128_sb = const.tile([128, 128], bf16)
            nc.sync.dma_start(i128_sb[:], i128_d[:])
            Wp_sb = const.tile([128, NKH * C], bf16)
            nc.sync.dma_start(
                Wp_sb[:].rearrange("p (k c) -> p k c", k=NKH),
                Wp_d.rearrange("k p c -> p k c"),
            )
            bp_sb = const.tile([BC, C], f32)
            nc.sync.dma_start(bp_sb[:], bp_d[:])
            ident = const.tile([128, 128], f32)
            make_identity(nc, ident[:])

            # state: hT bf16 [128, NKH*BC] (hT[:, 16k:16k+16] = h k-tile),
            # c fp32 [16, 512], both ping-pong.
            hT = [state.tile([128, NKH * BC], bf16, tag=f"hT{i}", name=f"hT{i}") for i in range(2)]
            cs = [state.tile([BC, H], f32, tag=f"c{i}", name=f"c{i}") for i in range(2)]

            xtiles, ptiles = {}, {}

            def dma_xtile(m):
                xtiles[m] = xring.tile([128, NKX * 128], bf16, tag="xt", name=f"xt{m}")
                nc.sync.dma_start(xtiles[m][:], xT_d[m])

            def phase1_chunk(m, jj):
                """One gate-chunk (512 cols) of m-tile m: 3 MMs + cast-copy."""
                if jj == 0:
                    ptiles[m] = pring.tile([128, G4], bf16, tag="proj", name=f"proj{m}")
                    if m + 1 < n_mt and m + 1 not in xtiles:
                        dma_xtile(m + 1)
                xt = xtiles[m]
                ps = p1ps.tile([128, 512], f32, tag="p1", name=f"p1_{m}_{jj}")
                for k in range(NKX):
                    nc.tensor.matmul(
                        ps[:],
                        lhsT=xt[:, k * 128:(k + 1) * 128],
                        rhs=Wx_sb[:, k * G4 + jj * 512: k * G4 + (jj + 1) * 512],
                        start=(k == 0),
                        stop=(k == NKX - 1),
                    )
                dst = ptiles[m][:, jj * 512:(jj + 1) * 512]
                if jj % 2 == 0:
                    nc.vector.tensor_copy(dst, ps[:])
                else:
                    nc.scalar.copy(dst, ps[:])

            def inject(t, banks):
                """Start each gate bank's accumulation with proj(+bias)."""
                m, tt = t // TPM, t % TPM
                for j in PROC:
                    nc.tensor.matmul(
                        banks[j][:],
                        lhsT=i128_sb[:, BC * tt:BC * (tt + 1)],
                        rhs=ptiles[m][:, j * 512:(j + 1) * 512],
                        start=True,
                        stop=(t == 0),
                    )

            # ---------------- preamble ----------------
            for m in range(min(LOOKAHEAD + 1, n_mt)):
                dma_xtile(m)
            for m in range(min(LOOKAHEAD, n_mt)):
                for jj in range(4):
                    phase1_chunk(m, jj)

            banks = [gbank.tile([BC, 512], f32, tag=f"gb{j}", name=f"gb{j}") for j in range(4)]
            inject(0, banks)

            p1_queue = []  # pending (m, jj) phase-1 chunks, 1 emitted per step
            for m in range(LOOKAHEAD, n_mt):
                for jj in range(4):
                    p1_queue.append((m, jj))

            qi = 0
            for t in range(s_total):
                ping = t % 2

                # h @ Wh accumulation, bank-major (f, g, i, o)
                if t > 0:
                    hpA = hTA[(t + 1) % 2]
                    hpB = hTB[(t + 1) % 2]
                    for j in PROC:
                        for k in range(NKH):
                            lh = (hpA[:, BC * k:BC * (k + 1)] if k < 2
                                  else hpB[:, BC * (k - 2):BC * (k - 1)])
                            nc.tensor.matmul(
                                banks[j][:],
                                lhsT=lh,
                                rhs=Wh_sb[:, k * G4 + j * 512: k * G4 + (j + 1) * 512],
                                start=False,
                                stop=(k == NKH - 1),
                            )

                # gate activations per bank, in processing order
                gf = work.tile([BC, H], f32, tag="gf", name="gf")
                nc.scalar.activation(gf[:], banks[2][:], AF.Sigmoid)
                gg = work.tile([BC, H], f32, tag="gg", name="gg")
                nc.scalar.activation(gg[:], banks[0][:], AF.Tanh)
                gi_ = work.tile([BC, H], f32, tag="gi", name="gi")
                nc.scalar.activation(gi_[:], banks[1][:], AF.Sigmoid)
                go = work.tile([BC, H], f32, tag="go", name="go")
                nc.scalar.activation(go[:], banks[3][:], AF.Sigmoid)

                # cell update (fp32): c = g*i + c*f
                prod = work.tile([BC, H], f32, tag="prod", name="prod")
                nc.vector.tensor_mul(prod[:], gg[:], gi_[:])
                if t > 0:
                    cf = work.tile([BC, H], f32, tag="cf", name="cf")
                    nc.vector.tensor_mul(cf[:], cs[(t + 1) % 2][:], gf[:])
                    nc.vector.tensor_add(cs[ping][:], prod[:], cf[:])
                else:
                    nc.vector.tensor_copy(cs[ping][:], prod[:])

                # prefill next step's banks (reuse same 4 PSUM tiles)
                if t + 1 < s_total:
                    banks = [gbank.tile([BC, 512], f32, tag=f"gb{j}", name=f"gb{j}_{t + 1}") for j in range(4)]
                    inject(t + 1, banks)

                # transposes: o then c quarters -> [128, BC] tiles
                oT = tbank.tile([128, NKH * BC], f32, tag="oT", name="oT")
                cT = tbank.tile([128, NKH * BC], f32, tag="cT", name="cT")
                for k in range(NKH):
                    nc.tensor.transpose(
                        oT[:, BC * k:BC * (k + 1)],
                        go[:, 128 * k:128 * (k + 1)],
                        ident[0:BC, 0:BC],
                    )
                for k in range(NKH):
                    nc.tensor.transpose(
                        cT[:, BC * k:BC * (k + 1)],
                        cs[ping][:, 128 * k:128 * (k + 1)],
                        ident[0:BC, 0:BC],
                    )
                # one phase-1 chunk per 2 steps fills the PE gap here
                if t % 2 == 0 and qi < len(p1_queue):
                    phase1_chunk(*p1_queue[qi])
                    qi += 1

                thT = work.tile([128, NKH * BC], f32, tag="thT", name="thT")
                nc.scalar.activation(thT[:], cT[:], AF.Tanh)
                nc.vector.tensor_mul(hT[ping][:], thT[:], oT[:])

            # ---------------- final projection ----------------
            fin = (s_total + 1) % 2
            pso = p1ps.tile([BC, C], f32, tag="p1", name="pso")
            for k in range(NKH):
                hfin = hTA[fin] if k < 2 else hTB[fin]
                nc.tensor.matmul(
                    pso[:],
                    lhsT=hfin[:, BC * (k % 2):BC * (k % 2 + 1)],
                    rhs=Wp_sb[:, k * C:(k + 1) * C],
                    start=(k == 0),
                    stop=(k == NKH - 1),
                )
            res = work.tile([BC, C], f32, tag="res", name="res")
            nc.vector.tensor_add(res[:], pso[:], bp_sb[:])
            nc.sync.dma_start(out_d[:], res[:])

    nc.compile()
    return nc


def _prep_shared_inputs(Wgx, Wix, Wfx, Wox, Wgh, Wih, Wfh, Woh, bg, bi, bf, bo, Wph, bp):
    import ml_dtypes
    bf16 = ml_dtypes.bfloat16
    Wx_all = np.concatenate([Wgx, Wix, Wfx, Wox], axis=1).astype(np.float32)  # [D, G4]
    b_all = np.concatenate([bg, bi, bf, bo]).astype(np.float32)               # [G4]
    Wh_all = np.concatenate([Wgh, Wih, Wfh, Woh], axis=1).astype(np.float32)  # [H, G4]

    Wx = np.zeros((NKX, 128, G4), dtype=np.float32)
    Wx[:D // 128] = Wx_all.reshape(D // 128, 128, G4)
    Wx[NKX - 1, 0, :] = b_all                     # bias row (pairs with ones-row)
    Wx = np.ascontiguousarray(Wx.transpose(1, 0, 2)).reshape(128, NKX * G4)
    Wh = np.ascontiguousarray(Wh_all.reshape(NKH, 128, G4).transpose(1, 0, 2)).reshape(128, NKH * G4)
    Wp = np.ascontiguousarray(Wph.reshape(NKH, 128, C))
    bpr = np.broadcast_to(bp.astype(np.float32), (BC, C)).copy()
    return (Wx.astype(bf16), Wh.astype(bf16), np.eye(128, dtype=np.float32).astype(bf16),
            Wp.astype(bf16), bpr)


def _prep_core_inputs(x, core, s_total):
    """xT[m, p, k*128 + col], col = tt*16 + b: stationary x tiles + ones row."""
    import ml_dtypes
    n_mt = s_total // TPM
    b0 = core * BC
    xc = np.asarray(x[b0:b0 + BC, :s_total, :], dtype=np.float32)   # [BC, s, D]
    a = np.ascontiguousarray(xc.transpose(2, 1, 0))                 # [D, s, BC]
    a = a.reshape(D // 128, 128, n_mt, TPM, BC)                     # [k, p, m, tt, b]
    a = a.transpose(2, 1, 0, 3, 4).reshape(n_mt, 128, D // 128, TPM * BC)
    xT = np.zeros((n_mt, 128, NKX, 128), dtype=np.float32)
    xT[:, :, :D // 128, :] = a
    xT[:, 0, NKX - 1, :] = 1.0                                      # ones row
    return {"xT": np.ascontiguousarray(xT).reshape(n_mt, 128, NKX * 128).astype(ml_dtypes.bfloat16)}


_NC_CACHE = {}


def _get_nc(s_total):
    if s_total not in _NC_CACHE:
        _NC_CACHE[s_total] = _build_nc(s_total)
    return _NC_CACHE[s_total]


def kernel(x, Wgx, Wix, Wfx, Wox, Wgh, Wih, Wfh, Woh, bg, bi, bf, bo, Wph, bp,
           _s_total=S, _trace=False, _trace_kwargs=None):
    from concourse import bass_utils

    x = np.asarray(x, dtype=np.float32)
    args = [np.asarray(a, dtype=np.float32) for a in
            (Wgx, Wix, Wfx, Wox, Wgh, Wih, Wfh, Woh, bg, bi, bf, bo, Wph, bp)]
    Wx, Wh, i16, Wp, bpr = _prep_shared_inputs(*args)

    nc = _get_nc(_s_total)
    in_maps = []
    for core in range(NCORES):
        m = _prep_core_inputs(x, core, _s_total)
        m.update({"Wx": Wx, "Wh": Wh, "ident128": i16, "Wp": Wp, "bp": bpr})
        in_maps.append(m)

    kw = {}
    if _trace:
        kw["trace"] = True
        kw.update(_trace_kwargs or {})
    res = bass_utils.run_bass_kernel_spmd(nc, in_maps, core_ids=list(range(NCORES)), **kw)
    out = np.concatenate([res.results[c]["out"] for c in range(NCORES)], axis=0)
    if _trace:
        kernel._last_results = res
    return out


def _sim_selftest(s_total=32, core=1):
    """CoreSim numerics check on one core vs numpy LSTM (no hardware)."""
    from concourse.bass_interp import CoreSim

    rng = np.random.default_rng(0)
    x = rng.standard_normal((B, s_total, D), dtype=np.float32)
    mk = lambda *s: (rng.standard_normal(s, dtype=np.float32) * 0.06)
    Wgx, Wix, Wfx, Wox = (mk(D, H) for _ in range(4))
    Wgh, Wih, Wfh, Woh = (mk(H, H) for _ in range(4))
    bg, bi, bf, bo = (rng.standard_normal(H).astype(np.float32) * 0.05 for _ in range(4))
    Wph = mk(H, C)
    bp = rng.standard_normal(C).astype(np.float32) * 0.05

    def ref_np(xc):
        sig = lambda v: 1.0 / (1.0 + np.exp(-v))
        h = np.zeros((xc.shape[0], H), np.float32)
        c = np.zeros((xc.shape[0], H), np.float32)
        for t in range(s_total):
            xt = xc[:, t, :]
            g = np.tanh(xt @ Wgx + bg + h @ Wgh)
            i = sig(xt @ Wix + bi + h @ Wih)
            f = sig(xt @ Wfx + bf + h @ Wfh)
            o = sig(xt @ Wox + bo + h @ Woh)
            c = g * i + c * f
            h = np.tanh(c) * o
        return h @ Wph + bp

    args = (Wgx, Wix, Wfx, Wox, Wgh, Wih, Wfh, Woh, bg, bi, bf, bo, Wph, bp)
    Wx, Wh, i16, Wp, bpr = _prep_shared_inputs(*args)
    nc = _build_nc(s_total)

    m = _prep_core_inputs(x, core, s_total)
    m.update({"Wx": Wx, "Wh": Wh, "ident128": i16, "Wp": Wp, "bp": bpr})

    sim = CoreSim(nc)
    for k, v in m.items():
        sim.tensor(k)[:] = v
    sim.simulate(check_with_hw=False)
    got = np.array(sim.tensor("out"))
    want = ref_np(x[core * BC:(core + 1) * BC])
    err = np.abs(got - want).max() / max(np.abs(want).max(), 1e-6)
    print(f"selftest S={s_total}: rel err {err:.3e}")
    assert err < 2e-2, err
    return err


if __name__ == "__main__":
    _sim_selftest(32)


# revision 22
# speedup vs baseline: 1.2560x; 1.0206x over previous
"""LSTM kernel for Trainium2 (Bass/Tile), SPMD over 8 NeuronCores.

Problem: B=128, S=1024, D=256, H=512, C=10 LSTM; output = final hidden state
projected to C classes -> [B, C].

Sharding: data-parallel over batch (16 per core); weights replicated;
recurrence local per shard (no collectives).

Per-core design (fused single pass; x read once, proj never leaves chip):
  - x-projections computed in m-tiles of 8 timesteps (M=128 = full PE
    width) a few steps ahead of the recurrence, result cast to bf16 into
    an SBUF ring -- no DRAM proj round-trip.
  - Per step, gate preactivations live in one PSUM tile [16, 2048]
    (4 banks: g|i|f|o). Each bank's accumulation group: an identity
    matmul injects proj (+bias, folded via phase-1's ones-row) with
    start=True, then 4 h @ Wh matmuls accumulate (bf16, N=512).
  - Banks are processed f,g,i,o so ACT/DVE for early gates overlap the
    later banks' matmuls; o last (shortest post-MM path).
  - Tail: cell update in bf16 [16, 512] (DVE 2x mode); then PE-transpose
    c and o quarters and compute hT = tanh(cT) * oT directly in the
    stationary layout (no h transpose). The c/thT/hT tensors are split
    into H-halves (A = k0,k1 / B = k2,k3) in separate tiles/PSUM banks so
    ho_A releases the next step's first two k-tile matmuls while the B
    half of the tail is still finishing.
"""

import numpy as np

S, B, D, H, C = 1024, 128, 256, 512, 10
NCORES = 8
BC = B // NCORES          # batch per core (16)
G4 = 4 * H                # fused gate width (2048)
NKH = H // 128            # 4 k-tiles for h
NKX = D // 128 + 1        # 2 k-tiles for x + 1 bias(ones) tile
TPM = 8                   # timesteps per phase-1 m-tile (128/BC)
LOOKAHEAD = 2             # m-tiles of proj lookahead
# gate memory order [g,i,f,o]; processing order f,g,i,o (o last)
PROC = [2, 0, 1, 3]


def _build_nc(s_total: int):
    import concourse.bass as bass
    import concourse.mybir as mybir
    import concourse.tile as tile
    from concourse import bacc
    from concourse.masks import make_identity

    f32 = mybir.dt.float32
    bf16 = mybir.dt.bfloat16
    AF = mybir.ActivationFunctionType

    n_mt = (s_total + TPM - 1) // TPM
    assert s_total % TPM == 0

    nc = bacc.Bacc(
        "TRN2",
        target_bir_lowering=False,
        debug=False,
        enable_asserts=False,
        num_devices=NCORES,
    )

    xT_d = nc.dram_tensor("xT", [n_mt, 128, NKX * 128], bf16, kind="ExternalInput").ap()
    Wx_d = nc.dram_tensor("Wx", [128, NKX * G4], bf16, kind="ExternalInput").ap()
    Wh_d = nc.dram_tensor("Wh", [128, NKH * G4], bf16, kind="ExternalInput").ap()
    i128_d = nc.dram_tensor("ident128", [128, 128], bf16, kind="ExternalInput").ap()
    Wp_d = nc.dram_tensor("Wp", [NKH, 128, C], bf16, kind="ExternalInput").ap()
    bp_d = nc.dram_tensor("bp", [BC, C], f32, kind="ExternalInput").ap()
    out_d = nc.dram_tensor("out", [BC, C], f32, kind="ExternalOutput").ap()

    with tile.TileContext(nc) as tc:
        with (
            tc.tile_pool(name="const", bufs=1) as const,
            tc.tile_pool(name="state", bufs=1) as state,
            tc.tile_pool(name="xring", bufs=3) as xring,
            tc.tile_pool(name="pring", bufs=LOOKAHEAD + 2) as pring,
            tc.tile_pool(name="p1ps", bufs=1, space="PSUM") as p1ps,
            tc.tile_pool(name="gbank", bufs=1, space="PSUM") as gbank,
            tc.tile_pool(name="tbank", bufs=1, space="PSUM") as tbank,
            tc.tile_pool(name="work", bufs=2) as work,
        ):
            Wx_sb = const.tile([128, NKX * G4], bf16)
            nc.sync.dma_start(Wx_sb[:], Wx_d[:])
            Wh_sb = const.tile([128, NKH * G4], bf16)
            nc.sync.dma_start(Wh_sb[:], Wh_d[:])
            i128_sb = const.tile([128, 128], bf16)
            nc.sync.dma_start(i128_sb[:], i128_d[:])
            Wp_sb = const.tile([128, NKH * C], bf16)
            nc.sync.dma_start(
                Wp_sb[:].rearrange("p (k c) -> p k c", k=NKH),
                Wp_d.rearrange("k p c -> p k c"),
            )
            bp_sb = const.tile([BC, C], f32)
            nc.sync.dma_start(bp_sb[:], bp_d[:])
            ident = const.tile([128, 128], f32)
            make_identity(nc, ident[:])

            # state: hT bf16 [128, NKH*BC] (hT[:, 16k:16k+16] = h k-tile),
            # c fp32 [16, 512], both ping-pong.
            hT = [state.tile([128, NKH * BC], bf16, tag=f"hT{i}", name=f"hT{i}") for i in range(2)]
            cs = [state.tile([BC, H], f32, tag=f"c{i}", name=f"c{i}") for i in range(2)]

            xtiles, ptiles = {}, {}

            def dma_xtile(m):
                xtiles[m] = xring.tile([128, NKX * 128], bf16, tag="xt", name=f"xt{m}")
                nc.sync.dma_start(xtiles[m][:], xT_d[m])

            def phase1_chunk(m, jj):
                """One gate-chunk (512 cols) of m-tile m: 3 MMs + cast-copy."""
                if jj == 0:
                    ptiles[m] = pring.tile([128, G4], bf16, tag="proj", name=f"proj{m}")
                    if m + 1 < n_mt and m + 1 not in xtiles:
                        dma_xtile(m + 1)
                xt = xtiles[m]
                ps = p1ps.tile([128, 512], f32, tag="p1", name=f"p1_{m}_{jj}")
                for k in range(NKX):
                    nc.tensor.matmul(
                        ps[:],
                        lhsT=xt[:, k * 128:(k + 1) * 128],
                        rhs=Wx_sb[:, k * G4 + jj * 512: k * G4 + (jj + 1) * 512],
                        start=(k == 0),
                        stop=(k == NKX - 1),
                    )
                dst = ptiles[m][:, jj * 512:(jj + 1) * 512]
                if jj % 2 == 0:
                    nc.vector.tensor_copy(dst, ps[:])
                else:
                    nc.scalar.copy(dst, ps[:])

            def inject(t, banks):
                """Start each gate bank's accumulation with proj(+bias)."""
                m, tt = t // TPM, t % TPM
                for j in PROC:
                    nc.tensor.matmul(
                        banks[j][:],
                        lhsT=i128_sb[:, BC * tt:BC * (tt + 1)],
                        rhs=ptiles[m][:, j * 512:(j + 1) * 512],
                        start=True,
                        stop=(t == 0),
                    )

            # ---------------- preamble ----------------
            for m in range(min(LOOKAHEAD + 1, n_mt)):
                dma_xtile(m)
            for m in range(min(LOOKAHEAD, n_mt)):
                for jj in range(4):
                    phase1_chunk(m, jj)

            banks = [gbank.tile([BC, 512], f32, tag=f"gb{j}", name=f"gb{j}") for j in range(4)]
            inject(0, banks)

            p1_queue = []  # pending (m, jj) phase-1 chunks, 1 emitted per step
            for m in range(LOOKAHEAD, n_mt):
                for jj in range(4):
                    p1_queue.append((m, jj))

            qi = 0
            for t in range(s_total):
                ping = t % 2

                # h @ Wh accumulation, bank-major (f, g, i, o)
                if t > 0:
                    hpA = hTA[(t + 1) % 2]
                    hpB = hTB[(t + 1) % 2]
                    for j in PROC:
                        for k in range(NKH):
                            lh = (hpA[:, BC * k:BC * (k + 1)] if k < 2
                                  else hpB[:, BC * (k - 2):BC * (k - 1)])
                            nc.tensor.matmul(
                                banks[j][:],
                                lhsT=lh,
                                rhs=Wh_sb[:, k * G4 + j * 512: k * G4 + (j + 1) * 512],
                                start=False,
                                stop=(k == NKH - 1),
                            )

                # gate activations per bank, in processing order
                gf = work.tile([BC, H], f32, tag="gf", name="gf")
                nc.scalar.activation(gf[:], banks[2][:], AF.Sigmoid)
                gg = work.tile([BC, H], f32, tag="gg", name="gg")
                nc.scalar.activation(gg[:], banks[0][:], AF.Tanh)
                gi_ = work.tile([BC, H], f32, tag="gi", name="gi")
                nc.scalar.activation(gi_[:], banks[1][:], AF.Sigmoid)
                go = work.tile([BC, H], f32, tag="go", name="go")
                nc.scalar.activation(go[:], banks[3][:], AF.Sigmoid)

                # cell update (bf16, H-halved so the A-half finishes early):
                # c = g*i + c*f
                HH = H // 2
                if t > 0:
                    cfA = work.tile([BC, HH], bf16, tag="cfA", name="cfA")
                    nc.vector.tensor_mul(cfA[:], csA[(t + 1) % 2][:], gf[:, 0:HH])
                    cfB = work.tile([BC, HH], bf16, tag="cfB", name="cfB")
                    nc.vector.tensor_mul(cfB[:], csB[(t + 1) % 2][:], gf[:, HH:H])
                    prodA = work.tile([BC, HH], bf16, tag="prodA", name="prodA")
                    nc.vector.tensor_mul(prodA[:], gg[:, 0:HH], gi_[:, 0:HH])
                    nc.vector.tensor_add(csA[ping][:], prodA[:], cfA[:])
                    prodB = work.tile([BC, HH], bf16, tag="prodB", name="prodB")
                    nc.vector.tensor_mul(prodB[:], gg[:, HH:H], gi_[:, HH:H])
                    nc.vector.tensor_add(csB[ping][:], prodB[:], cfB[:])
                else:
                    nc.vector.tensor_mul(csA[ping][:], gg[:, 0:HH], gi_[:, 0:HH])
                    nc.vector.tensor_mul(csB[ping][:], gg[:, HH:H], gi_[:, HH:H])

                # prefill next step's banks (reuse same 4 PSUM tiles)
                if t + 1 < s_total:
                    banks = [gbank.tile([BC, 512], f32, tag=f"gb{j}", name=f"gb{j}_{t + 1}") for j in range(4)]
                    inject(t + 1, banks)

                # transposes: o then c quarters -> [128, BC] tiles
                oT = tbank.tile([128, NKH * BC], f32, tag="oT", name="oT")
                cT = tbank.tile([128, NKH * BC], f32, tag="cT", name="cT")
                for k in range(NKH):
                    nc.tensor.transpose(
                        oT[:, BC * k:BC * (k + 1)],
                        go[:, 128 * k:128 * (k + 1)],
                        ident[0:BC, 0:BC],
                    )
                for k in range(NKH):
                    nc.tensor.transpose(
                        cT[:, BC * k:BC * (k + 1)],
                        cs[ping][:, 128 * k:128 * (k + 1)],
                        ident[0:BC, 0:BC],
                    )
                # one phase-1 chunk per 2 steps fills the PE gap here
                if t % 2 == 0 and qi < len(p1_queue):
                    phase1_chunk(*p1_queue[qi])
                    qi += 1

                thT = work.tile([128, NKH * BC], f32, tag="thT", name="thT")
                nc.scalar.activation(thT[:], cT[:], AF.Tanh)
                nc.vector.tensor_mul(hT[ping][:], thT[:], oT[:])

            # ---------------- final projection ----------------
            fin = (s_total + 1) % 2
            pso = p1ps.tile([BC, C], f32, tag="p1", name="pso")
            for k in range(NKH):
                hfin = hTA[fin] if k < 2 else hTB[fin]
                nc.tensor.matmul(
                    pso[:],
                    lhsT=hfin[:, BC * (k % 2):BC * (k % 2 + 1)],
                    rhs=Wp_sb[:, k * C:(k + 1) * C],
                    start=(k == 0),
                    stop=(k == NKH - 1),
                )
            res = work.tile([BC, C], f32, tag="res", name="res")
            nc.vector.tensor_add(res[:], pso[:], bp_sb[:])
            nc.sync.dma_start(out_d[:], res[:])

    nc.compile()
    return nc


def _prep_shared_inputs(Wgx, Wix, Wfx, Wox, Wgh, Wih, Wfh, Woh, bg, bi, bf, bo, Wph, bp):
    import ml_dtypes
    bf16 = ml_dtypes.bfloat16
    Wx_all = np.concatenate([Wgx, Wix, Wfx, Wox], axis=1).astype(np.float32)  # [D, G4]
    b_all = np.concatenate([bg, bi, bf, bo]).astype(np.float32)               # [G4]
    Wh_all = np.concatenate([Wgh, Wih, Wfh, Woh], axis=1).astype(np.float32)  # [H, G4]

    Wx = np.zeros((NKX, 128, G4), dtype=np.float32)
    Wx[:D // 128] = Wx_all.reshape(D // 128, 128, G4)
    Wx[NKX - 1, 0, :] = b_all                     # bias row (pairs with ones-row)
    Wx = np.ascontiguousarray(Wx.transpose(1, 0, 2)).reshape(128, NKX * G4)
    Wh = np.ascontiguousarray(Wh_all.reshape(NKH, 128, G4).transpose(1, 0, 2)).reshape(128, NKH * G4)
    Wp = np.ascontiguousarray(Wph.reshape(NKH, 128, C))
    bpr = np.broadcast_to(bp.astype(np.float32), (BC, C)).copy()
    return (Wx.astype(bf16), Wh.astype(bf16), np.eye(128, dtype=np.float32).astype(bf16),
            Wp.astype(bf16), bpr)


def _prep_core_inputs(x, core, s_total):
    """xT[m, p, k*128 + col], col = tt*16 + b: stationary x tiles + ones row."""
    import ml_dtypes
    n_mt = s_total // TPM
    b0 = core * BC
    xc = np.asarray(x[b0:b0 + BC, :s_total, :], dtype=np.float32)   # [BC, s, D]
    a = np.ascontiguousarray(xc.transpose(2, 1, 0))                 # [D, s, BC]
    a = a.reshape(D // 128, 128, n_mt, TPM, BC)                     # [k, p, m, tt, b]
    a = a.transpose(2, 1, 0, 3, 4).reshape(n_mt, 128, D // 128, TPM * BC)
    xT = np.zeros((n_mt, 128, NKX, 128), dtype=np.float32)
    xT[:, :, :D // 128, :] = a
    xT[:, 0, NKX - 1, :] = 1.0                                      # ones row
    return {"xT": np.ascontiguousarray(xT).reshape(n_mt, 128, NKX * 128).astype(ml_dtypes.bfloat16)}


_NC_CACHE = {}


def _get_nc(s_total):
    if s_total not in _NC_CACHE:
        _NC_CACHE[s_total] = _build_nc(s_total)
    return _NC_CACHE[s_total]


def kernel(x, Wgx, Wix, Wfx, Wox, Wgh, Wih, Wfh, Woh, bg, bi, bf, bo, Wph, bp,
           _s_total=S, _trace=False, _trace_kwargs=None):
    from concourse import bass_utils

    x = np.asarray(x, dtype=np.float32)
    args = [np.asarray(a, dtype=np.float32) for a in
            (Wgx, Wix, Wfx, Wox, Wgh, Wih, Wfh, Woh, bg, bi, bf, bo, Wph, bp)]
    Wx, Wh, i16, Wp, bpr = _prep_shared_inputs(*args)

    nc = _get_nc(_s_total)
    in_maps = []
    for core in range(NCORES):
        m = _prep_core_inputs(x, core, _s_total)
        m.update({"Wx": Wx, "Wh": Wh, "ident128": i16, "Wp": Wp, "bp": bpr})
        in_maps.append(m)

    kw = {}
    if _trace:
        kw["trace"] = True
        kw.update(_trace_kwargs or {})
    res = bass_utils.run_bass_kernel_spmd(nc, in_maps, core_ids=list(range(NCORES)), **kw)
    out = np.concatenate([res.results[c]["out"] for c in range(NCORES)], axis=0)
    if _trace:
        kernel._last_results = res
    return out


def _sim_selftest(s_total=32, core=1):
    """CoreSim numerics check on one core vs numpy LSTM (no hardware)."""
    from concourse.bass_interp import CoreSim

    rng = np.random.default_rng(0)
    x = rng.standard_normal((B, s_total, D), dtype=np.float32)
    mk = lambda *s: (rng.standard_normal(s, dtype=np.float32) * 0.06)
    Wgx, Wix, Wfx, Wox = (mk(D, H) for _ in range(4))
    Wgh, Wih, Wfh, Woh = (mk(H, H) for _ in range(4))
    bg, bi, bf, bo = (rng.standard_normal(H).astype(np.float32) * 0.05 for _ in range(4))
    Wph = mk(H, C)
    bp = rng.standard_normal(C).astype(np.float32) * 0.05

    def ref_np(xc):
        sig = lambda v: 1.0 / (1.0 + np.exp(-v))
        h = np.zeros((xc.shape[0], H), np.float32)
        c = np.zeros((xc.shape[0], H), np.float32)
        for t in range(s_total):
            xt = xc[:, t, :]
            g = np.tanh(xt @ Wgx + bg + h @ Wgh)
            i = sig(xt @ Wix + bi + h @ Wih)
            f = sig(xt @ Wfx + bf + h @ Wfh)
            o = sig(xt @ Wox + bo + h @ Woh)
            c = g * i + c * f
            h = np.tanh(c) * o
        return h @ Wph + bp

    args = (Wgx, Wix, Wfx, Wox, Wgh, Wih, Wfh, Woh, bg, bi, bf, bo, Wph, bp)
    Wx, Wh, i16, Wp, bpr = _prep_shared_inputs(*args)
    nc = _build_nc(s_total)

    m = _prep_core_inputs(x, core, s_total)
    m.update({"Wx": Wx, "Wh": Wh, "ident128": i16, "Wp": Wp, "bp": bpr})

    sim = CoreSim(nc)
    for k, v in m.items():
        sim.tensor(k)[:] = v
    sim.simulate(check_with_hw=False)
    got = np.array(sim.tensor("out"))
    want = ref_np(x[core * BC:(core + 1) * BC])
    err = np.abs(got - want).max() / max(np.abs(want).max(), 1e-6)
    print(f"selftest S={s_total}: rel err {err:.3e}")
    assert err < 2e-2, err
    return err


if __name__ == "__main__":
    _sim_selftest(32)
